# revision 1
# baseline (speedup 1.0000x reference)
"""Masked bidirectional Chamfer distance (B=16, N=M=4096, D=2) on 8
Trainium2 NeuronCores.

Algorithm
---------
d2 = s2 + t2 - 2 s.t, so  min_m d2 = s2 - 2 * max_m (s.t - t2/2).
The PE computes psum[q, m] = s.t + h (h = -t2/2, or a large negative
sentinel for padding) as a K=11 matmul of fp16 split pairs (2-term fp16
splits of each coordinate, 3-term split of h), which runs at the full
1 cycle/row PE rate while retaining ~fp32 precision.  VectorE reduce_max
collapses PSUM tiles to per-query running maxima; a tiny epilogue does
d2 = bias - 2 r, relu, sqrt (+1 Newton step).

Work layout: both clouds are sorted by x on the host (valid points only).
Each *unit* is one tile of 128 sorted queries paired with a width-W=512
window of sorted targets centred on the tile.  All 2*B directions' units
are dealt round-robin across the 8 cores, so every core executes the
identical program (G units) on host-packed buffers — perfect SPMD balance.
Two units share one 2-bank PSUM tile and a single 3D-AP reduce.  The host
post-verifies the window bound per query (NN distance <= x-gap to the
nearest excluded sorted target) and patches the rare violators (~0.2%)
with an exact numpy computation.
"""

import numpy as np

B, N, M = 16, 4096, 4096
NCORES = 8
K = 11               # split-matmul contraction rows
W = 512              # sorted-target window per unit
TILE = 128           # queries per unit
PAIR = 2             # units per PSUM tile / reduce
SENT = np.float16(-60000.0)   # x3 rows -> -180000 additive sentinel
BIG = 1e10

_CACHE = {}


# ----------------------------------------------------------------- host math
def _split16(x, n):
    out = []
    r = np.asarray(x, np.float64)
    for _ in range(n):
        h = r.astype(np.float16)
        out.append(h)
        r = r - h.astype(np.float64)
    return out


def _stat_rows(cloud):
    """[11, n] fp16 stationary rows for query points [n, 2]."""
    x1, x2 = _split16(cloud[:, 0], 2)
    y1, y2 = _split16(cloud[:, 1], 2)
    ones = np.ones(cloud.shape[0], np.float16)
    return np.stack([x1, x2, x1, x2, y1, y2, y1, y2, ones, ones, ones])


def _mov_rows(cloud):
    """[11, n] fp16 moving rows for target points [n, 2] (all valid)."""
    x1, x2 = _split16(cloud[:, 0], 2)
    y1, y2 = _split16(cloud[:, 1], 2)
    h = -0.5 * (cloud[:, 0].astype(np.float64) ** 2
                + cloud[:, 1].astype(np.float64) ** 2)
    h1, h2, h3 = _split16(h, 3)
    return np.stack([x1, x1, x2, x2, y1, y1, y2, y2, h1, h2, h3])


# ------------------------------------------------------------- device program
def _build_program(G, loop_n=None):
    """G units (must be a multiple of PAIR). loop_n wraps the body in a
    hardware loop executing it loop_n times (benchmarking only)."""
    import concourse.bacc as bacc
    import concourse.tile as tile
    from concourse import mybir
    from contextlib import ExitStack

    f32 = mybir.dt.float32
    f16 = mybir.dt.float16
    Alu = mybir.AluOpType
    Act = mybir.ActivationFunctionType
    assert G % PAIR == 0

    nc = bacc.Bacc()
    statbuf = nc.declare_dram_parameter("statbuf", (K, G * TILE), f16, isOutput=False)
    movbuf = nc.declare_dram_parameter("movbuf", (G // PAIR, K, PAIR, W), f16, isOutput=False)
    biasbuf = nc.declare_dram_parameter("biasbuf", (TILE, G), f32, isOutput=False)
    yout = nc.declare_dram_parameter("yout", (TILE, G), f32, isOutput=True)

    with ExitStack() as ctx:
        tc = ctx.enter_context(tile.TileContext(nc))
        singles = ctx.enter_context(tc.tile_pool(name="singles", bufs=1))
        movp = ctx.enter_context(tc.tile_pool(name="movp", bufs=6))
        psp = ctx.enter_context(tc.tile_pool(name="psp", bufs=4, space="PSUM"))
        epi = ctx.enter_context(tc.tile_pool(name="epi", bufs=1))

        def body():
            stat = singles.tile([K, G * TILE], f16, tag="stat")
            nc.sync.dma_start(out=stat, in_=statbuf.ap())
            bias = singles.tile([TILE, G], f32, tag="bias")
            nc.sync.dma_start(out=bias, in_=biasbuf.ap())
            rstage = singles.tile([TILE, G], f32, tag="rstage")

            for p in range(G // PAIR):
                g0 = p * PAIR
                mov = movp.tile([K, PAIR, W], f16, tag="mov")
                nc.sync.dma_start(out=mov, in_=movbuf[p])
                ps = psp.tile([TILE, PAIR * W], f32, tag="ps")
                for i in range(PAIR):
                    g = g0 + i
                    nc.tensor.matmul(
                        ps[:, i * W : (i + 1) * W],
                        lhsT=stat[:, g * TILE : (g + 1) * TILE],
                        rhs=mov[:, i, :], start=True, stop=True)
                nc.vector.reduce_max(
                    rstage[:, g0 : g0 + PAIR],
                    ps.rearrange("p (u w) -> p u w", u=PAIR),
                    axis=mybir.AxisListType.X)

            # epilogue on [TILE, G]: y = sqrt(max(bias - 2 r, eps)) + Newton
            d2 = epi.tile([TILE, G], f32, tag="d2")
            nc.vector.scalar_tensor_tensor(out=d2, in0=rstage, scalar=-2.0,
                                           in1=bias, op0=Alu.mult, op1=Alu.add)
            nc.vector.tensor_scalar_max(out=d2, in0=d2, scalar1=1e-30)
            y0 = epi.tile([TILE, G], f32, tag="y0")
            nc.scalar.activation(out=y0, in_=d2, func=Act.Sqrt)
            rc = epi.tile([TILE, G], f32, tag="rc")
            nc.vector.reciprocal(out=rc, in_=y0)
            t1 = epi.tile([TILE, G], f32, tag="t1")
            nc.vector.tensor_mul(t1, d2, rc)
            ysum = epi.tile([TILE, G], f32, tag="ysum")
            nc.vector.tensor_add(ysum, y0, t1)
            outt = epi.tile([TILE, G], f32, tag="outt")
            nc.vector.tensor_scalar_mul(out=outt, in0=ysum, scalar1=0.5)
            nc.sync.dma_start(out=yout.ap(), in_=outt[:])

        if loop_n is None:
            body()
        else:
            with tc.For_i(0, loop_n, 1):
                body()
    nc.finalize()
    return nc


# --------------------------------------------------------------- host driver
def _prepare(sc, tcl, sm, tm):
    """Build per-core packed buffers and the unit bookkeeping table."""
    jobs = []
    for b in range(B):
        for d, (q, t, qlen, tlen) in enumerate(
            ((sc[b], tcl[b], sm[b], tm[b]), (tcl[b], sc[b], tm[b], sm[b]))
        ):
            if qlen == 0:
                continue  # no valid queries; outputs stay 0
            qv = q[:qlen]
            qidx = np.argsort(qv[:, 0], kind="stable")
            qs = qv[qidx]
            stat11 = _stat_rows(qs)
            s2 = (qs[:, 0] * qs[:, 0] + qs[:, 1] * qs[:, 1]).astype(np.float32)
            if tlen > 0:
                tv = t[:tlen]
                ts = tv[np.argsort(tv[:, 0], kind="stable")]
                mov11 = _mov_rows(ts)
                tx = ts[:, 0].astype(np.float64)
            else:
                mov11 = np.zeros((K, 0), np.float16)
                tx = np.zeros(0, np.float64)
            jobs.append(dict(b=b, d=d, qlen=int(qlen), tlen=int(tlen),
                             qidx=qidx, qs=qs, stat11=stat11, s2=s2,
                             mov11=mov11, tx=tx))

    units = []
    for j in jobs:
        ntiles = (j["qlen"] + TILE - 1) // TILE
        for k in range(ntiles):
            units.append((j, k))

    per_core = (len(units) + NCORES - 1) // NCORES
    G = ((per_core + PAIR - 1) // PAIR) * PAIR
    sent_col = np.zeros((K, 1), np.float16)
    sent_col[8:11, 0] = SENT

    in_maps = []
    meta = []
    for c in range(NCORES):
        statpack = np.zeros((K, G * TILE), np.float16)
        movpack = np.tile(np.broadcast_to(sent_col, (K, W)).copy()[None, :, None],
                          (G // PAIR, 1, PAIR, 1))
        biaspack = np.zeros((TILE, G), np.float32)
        cmeta = []
        for g in range(G):
            # deal units round-robin by core so trailing padding lands on
            # the last cores only
            u = c + g * NCORES
            if u >= len(units):
                cmeta.append(None)
                continue
            j, k = units[u]
            lo_q = k * TILE
            hi_q = min(j["qlen"], lo_q + TILE)
            nq = hi_q - lo_q
            statpack[:, g * TILE : g * TILE + nq] = j["stat11"][:, lo_q:hi_q]
            biaspack[:nq, g] = j["s2"][lo_q:hi_q]
            tlen = j["tlen"]
            if tlen > 0:
                med = np.median(j["qs"][lo_q:hi_q, 0])
                ctr = np.searchsorted(j["tx"], med)
                lo = int(np.clip(ctr - W // 2, 0, max(0, tlen - W)))
                hi = min(tlen, lo + W)
                movpack[g // PAIR, :, g % PAIR, : hi - lo] = j["mov11"][:, lo:hi]
            else:
                lo = hi = 0
            cmeta.append(dict(j=j, k=k, nq=nq, lo=lo, hi=hi))
        in_maps.append({"statbuf": statpack, "movbuf": movpack,
                        "biasbuf": biaspack})
        meta.append(cmeta)
    return in_maps, meta, G


def _assemble(results, meta):
    fwd = np.zeros((B, N), np.float32)
    bwd = np.zeros((B, M), np.float32)
    outs = (fwd, bwd)
    patch = []
    for c in range(NCORES):
        y = results[c]["yout"]  # [TILE, G]
        for g, u in enumerate(meta[c]):
            if u is None:
                continue
            j = u["j"]
            vals = y[: u["nq"], g]
            rows = j["qidx"][u["k"] * TILE : u["k"] * TILE + u["nq"]]
            outs[j["d"]][j["b"], rows] = vals
            patch.append((j, u, rows, vals))
    return fwd, bwd, patch


def _verify_and_patch(fwd, bwd, patch, sc, tcl):
    """Window-bound check per query; exact recompute for violators and for
    empty-target jobs."""
    outs = (fwd, bwd)
    n_patched = 0
    for j, u, rows, vals in patch:
        tlen = j["tlen"]
        b, d = j["b"], j["d"]
        if tlen == 0:
            outs[d][b, rows] = np.float32(np.sqrt(BIG))
            continue
        qx = j["qs"][u["k"] * TILE : u["k"] * TILE + u["nq"], 0].astype(np.float64)
        gap = np.full(u["nq"], np.inf)
        if u["lo"] > 0:
            gap = np.minimum(gap, np.abs(qx - j["tx"][u["lo"] - 1]))
        if u["hi"] < tlen:
            gap = np.minimum(gap, np.abs(qx - j["tx"][u["hi"]]))
        bad = vals.astype(np.float64) + 2e-3 > gap
        if not bad.any():
            continue
        qpts = j["qs"][u["k"] * TILE : u["k"] * TILE + u["nq"]][bad]
        t = (tcl if d == 0 else sc)[b][:tlen].astype(np.float32)
        q2 = (qpts[:, 0] ** 2 + qpts[:, 1] ** 2).astype(np.float32)
        t2 = (t[:, 0] ** 2 + t[:, 1] ** 2).astype(np.float32)
        dot = qpts.astype(np.float32) @ t.T
        d2v = q2[:, None] + t2[None, :] - 2.0 * dot
        mn = np.maximum(d2v.min(axis=1), 0.0)
        outs[d][b, rows[bad]] = np.sqrt(mn).astype(np.float32)
        n_patched += int(bad.sum())
    return n_patched


def _get_program(G):
    key = ("nc", G)
    if key not in _CACHE:
        _CACHE[key] = _build_program(G)
    return _CACHE[key]


def kernel(source_cloud, target_cloud, source_mask, target_mask):
    from concourse.bass_utils import run_bass_kernel_spmd

    sc = np.ascontiguousarray(np.asarray(source_cloud, np.float32))
    tcl = np.ascontiguousarray(np.asarray(target_cloud, np.float32))
    sm = np.asarray(source_mask).astype(np.int64)
    tm = np.asarray(target_mask).astype(np.int64)

    in_maps, meta, G = _prepare(sc, tcl, sm, tm)
    if G == 0:
        # no valid queries anywhere: all outputs are zero
        return np.zeros((B, N), np.float32), np.zeros((B, M), np.float32)
    nc = _get_program(G)
    res = run_bass_kernel_spmd(nc, in_maps, list(range(NCORES)))
    fwd, bwd, patch = _assemble(res.results, meta)
    _verify_and_patch(fwd, bwd, patch, sc, tcl)
    return fwd, bwd

